# revision 5
# baseline (speedup 1.0000x reference)
"""Multi-head attention TRN2 kernel, 8-core SPMD.

Sharding: core c -> (batch b = c//2, head-group g = c%2 covering heads g*8..g*8+8).
Per core: QKV projections for its 8 heads, attention, partial output projection.
Host: sums the 2 out-proj partials per batch, scatters packed attention weights.

All value matmuls run in float32r (1 cyc/row, ~1.5e-4 rel err, HW-validated).
Mask applied via bf16 (-1e12 * I) matmuls into PSUM. Softmax without max-subtraction
(scores are O(1); masked entries underflow to exact 0 like the fp32 reference).
P normalized via second exp with bias=-lnZ; transposed path (PT, for P@V) normalized
via a rank-1 ones x (-lnZ_row) matmul added into S^T before its exp.

Mask is triaged on host at 128x128 block granularity: FREE (no mask work),
MASK (mask matmul), SKIP (fully masked: not computed, not stored; host writes 0).
"""
import functools
import os

import numpy as np

import concourse.bacc as bacc
import concourse.mybir as mybir
from concourse.tile import TileContext
from concourse.bass_utils import run_bass_kernel_spmd
from concourse.masks import make_identity

B, S, D, H = 4, 1024, 1024, 16
HD = D // H            # 64
NCORES = 8
HPC = H // 2           # 8 heads per core
DG = HPC * HD          # 512 per-core head dims
NBLK = S // 128        # 8 blocks of 128
f32 = mybir.dt.float32
f32r = mybir.dt.float32r
bf16 = mybir.dt.bfloat16
Exp = mybir.ActivationFunctionType.Exp
Ln = mybir.ActivationFunctionType.Ln
Copy = mybir.ActivationFunctionType.Copy

FREE, MASK, SKIP = 0, 1, 2

STATS = {}  # exec_time_ns etc. for test harness


def _classify_mask(mask2d):
    """Per 128x128 block: FREE (all zero), SKIP (fully masking), MASK (mixed).

    Returns (classes [8,8] int, cmaxb [8] int) with cmaxb monotone nondecreasing;
    non-suffix SKIPs are reclassified as MASK so storage is a per-row prefix.
    """
    m = mask2d.reshape(NBLK, 128, NBLK, 128)
    classes = np.empty((NBLK, NBLK), np.int64)
    for rb in range(NBLK):
        for cb in range(NBLK):
            blk = m[rb, :, cb, :]
            if not blk.any():
                classes[rb, cb] = FREE
            elif float(blk.min()) * 1e12 >= 1e10:
                classes[rb, cb] = SKIP
            else:
                classes[rb, cb] = MASK
    cmaxb = np.empty(NBLK, np.int64)
    prev = 0
    for rb in range(NBLK):
        nonskip = [cb for cb in range(NBLK) if classes[rb, cb] != SKIP]
        cm = (max(nonskip) + 1) if nonskip else 0
        cm = max(cm, prev, 1)  # monotone; compute at least one block
        prev = cm
        cmaxb[rb] = cm
        for cb in range(cm):
            if classes[rb, cb] == SKIP:
                classes[rb, cb] = MASK  # inside stored prefix: mask it to exact 0
    return classes, cmaxb


def _chunks(nblocks):
    """512-wide chunk list [(off, width)] covering nblocks*128 columns."""
    out = []
    off = 0
    total = nblocks * 128
    while off < total:
        w = min(512, total - off)
        out.append((off, w))
        off += w
    return out


def _rchunks(rmin_blk):
    """512-aligned chunks covering rows [rmin_blk*128, S)."""
    start = (rmin_blk * 128 // 512) * 512
    return [(off, 512) for off in range(start, S, 512)]


@functools.lru_cache(maxsize=4)
def _build_program(classes_key, qbias, kbias, vbias):
    classes = np.frombuffer(classes_key, dtype=np.int64).reshape(NBLK, NBLK)
    cmaxb = np.array([max(cb for cb in range(NBLK) if classes[rb, cb] != SKIP) + 1
                      for rb in range(NBLK)], np.int64)
    # rminb[cb]: first rb whose stored prefix includes cb (monotone cmaxb => suffix)
    rminb = np.array([min([rb for rb in range(NBLK) if cmaxb[rb] > cb] or [NBLK])
                      for cb in range(NBLK)], np.int64)
    offs = np.concatenate([[0], np.cumsum(cmaxb * 128 * 128)])
    packed = int(offs[-1])  # floats per head

    nc = bacc.Bacc(None, target_bir_lowering=False)
    xq_d = nc.declare_dram_parameter("xq", [S, D], f32, isOutput=False)
    xk_d = nc.declare_dram_parameter("xk", [S, D], f32, isOutput=False)
    xv_d = nc.declare_dram_parameter("xv", [S, D], f32, isOutput=False)
    m_d = nc.declare_dram_parameter("mask", [S, S], f32, isOutput=False)
    wq_d = nc.declare_dram_parameter("wq", [D, DG], f32, isOutput=False)
    wk_d = nc.declare_dram_parameter("wk", [D, DG], f32, isOutput=False)
    wv_d = nc.declare_dram_parameter("wv", [D, DG], f32, isOutput=False)
    wo_d = nc.declare_dram_parameter("wo", [DG, D], f32, isOutput=False)
    bq_d = nc.declare_dram_parameter("bq", [1, DG], f32, isOutput=False)
    bk_d = nc.declare_dram_parameter("bk", [1, DG], f32, isOutput=False)
    bv_d = nc.declare_dram_parameter("bv", [1, DG], f32, isOutput=False)
    p_d = nc.declare_dram_parameter("p", [HPC, packed], f32, isOutput=True)
    o_d = nc.declare_dram_parameter("o", [S, D], f32, isOutput=True)

    with TileContext(nc) as tc:
        with tc.tile_pool(name="const", bufs=1) as constp, \
             tc.tile_pool(name="persist", bufs=1) as pers:
            identF = constp.tile([128, 128], f32)
            make_identity(nc, identF[:])
            ident = constp.tile([128, 128], f32r)
            nc.vector.tensor_copy(out=ident[:], in_=identF[:])
            negI = constp.tile([128, 128], bf16)
            nc.gpsimd.memset(negI[:], 0.0)
            nc.gpsimd.affine_select(out=negI[:], in_=negI[:],
                                    compare_op=mybir.AluOpType.not_equal,
                                    fill=-1e12, base=0, pattern=[[-1, 128]],
                                    channel_multiplier=1)
            onesF = constp.tile([1, S], f32)
            nc.gpsimd.memset(onesF[:], 1.0)
            ones_row = constp.tile([1, S], f32r)
            nc.vector.tensor_copy(out=ones_row[:], in_=onesF[:])

            # persistent operands
            QT = pers.tile([128, 4, S], f32r)     # [p, pair j, s]; head h at rows (h%2)*64, pair h//2
            KT = pers.tile([128, 4, S], f32r)
            V = pers.tile([128, NBLK, DG], f32r)  # [s_in, s_blk, dcat]
            mask_t = pers.tile([128, NBLK, S], bf16)
            wo_p = pers.tile([64, HPC, D], f32r)  # [d_in_head, head, dout]
            nc.gpsimd.dma_start(out=mask_t[:],
                                in_=m_d.rearrange("(b p) c -> p b c", p=128))
            nc.gpsimd.dma_start(out=wo_p[:],
                                in_=wo_d.rearrange("(h p) n -> p h n", p=64))
            if qbias or kbias or vbias:
                b_rows = pers.tile([1, 3, DG], f32r)
                nc.gpsimd.dma_start(out=b_rows[0:1, 0, :], in_=bq_d[:])
                nc.gpsimd.dma_start(out=b_rows[0:1, 1, :], in_=bk_d[:])
                nc.gpsimd.dma_start(out=b_rows[0:1, 2, :], in_=bv_d[:])

            # ---------------- phase 1: load + transpose + projections ----------------
            with tc.tile_pool(name="p1", bufs=3) as p1, \
                 tc.tile_pool(name="p1w", bufs=1) as p1w, \
                 tc.tile_pool(name="p1xT", bufs=1) as p1xT, \
                 tc.tile_pool(name="ps1", bufs=2, space="PSUM") as ps1:
                for x_d, w_d, kind, has_b, bidx in [
                        (xq_d, wq_d, "q", qbias, 0),
                        (xk_d, wk_d, "k", kbias, 1),
                        (xv_d, wv_d, "v", vbias, 2)]:
                    w_t = p1w.tile([128, 8, DG], f32r, tag="w")
                    nc.gpsimd.dma_start(out=w_t[:],
                                        in_=w_d.rearrange("(i p) o -> p i o", p=128))
                    xT = p1xT.tile([128, 8, S], f32r, tag="xT")
                    for sb in range(NBLK):
                        x_t = p1.tile([128, D], f32r, tag="x")
                        nc.gpsimd.dma_start(out=x_t[:], in_=x_d[sb * 128:(sb + 1) * 128, :])
                        for ib in range(8):
                            tp = ps1.tile([128, 128], f32r, tag="tp")
                            nc.tensor.transpose(tp[:], x_t[:, ib * 128:(ib + 1) * 128], ident[:])
                            nc.vector.tensor_copy(out=xT[:, ib, sb * 128:(sb + 1) * 128],
                                                  in_=tp[:])
                    if kind in ("q", "k"):
                        dst = QT if kind == "q" else KT
                        for j in range(4):
                            for ch in range(2):
                                ps = ps1.tile([128, 512], f32, tag="proj")
                                for ki in range(8):
                                    nc.tensor.matmul(
                                        ps[:], w_t[:, ki, j * 128:(j + 1) * 128],
                                        xT[:, ki, ch * 512:(ch + 1) * 512],
                                        start=(ki == 0), stop=(ki == 7 and not has_b))
                                if has_b:
                                    nc.tensor.matmul(
                                        ps[:], b_rows[0:1, bidx, j * 128:(j + 1) * 128],
                                        ones_row[0:1, 0:512], start=False, stop=True)
                                nc.vector.tensor_copy(
                                    out=dst[:, j, ch * 512:(ch + 1) * 512], in_=ps[:])
                    else:
                        for sb in range(NBLK):
                            ps = ps1.tile([128, 512], f32, tag="proj")
                            for ki in range(8):
                                nc.tensor.matmul(
                                    ps[:], xT[:, ki, sb * 128:(sb + 1) * 128],
                                    w_t[:, ki, :],
                                    start=(ki == 0), stop=(ki == 7 and not has_b))
                            if has_b:
                                nc.tensor.matmul(ps[:], ones_row[0:1, 0:128],
                                                 b_rows[0:1, bidx, :],
                                                 start=False, stop=True)
                            nc.vector.tensor_copy(out=V[:, sb, :], in_=ps[:])

            # ---------------- phase 2: attention per head ----------------
            with tc.tile_pool(name="p2", bufs=3) as p2, \
                 tc.tile_pool(name="p2pn", bufs=2) as p2pn, \
                 tc.tile_pool(name="p2sm", bufs=3) as p2sm, \
                 tc.tile_pool(name="psS", bufs=3, space="PSUM") as psS, \
                 tc.tile_pool(name="psT", bufs=3, space="PSUM") as psT, \
                 tc.tile_pool(name="psA", bufs=2, space="PSUM") as psA:
                attnT = p2.tile([64, HPC, S], f32r, tag="attnT", bufs=1)
                for h in range(HPC):
                    hp, hq = h // 2, (h % 2) * 64
                    neglnz = p2sm.tile([128, NBLK], f32r, tag="neglnz")
                    # ---- natural path: S, Z, normalized P out ----
                    for rb in range(NBLK):
                        cm = int(cmaxb[rb])
                        chs = _chunks(cm)
                        sps_list = []
                        zts = []
                        pn = p2pn.tile([128, 1024], f32, tag="pn")
                        for (off, w) in chs:
                            sps = psS.tile([128, 512], f32, tag="S")
                            sps_list.append((sps, off, w))
                            mask_cbs = [cb for cb in range(off // 128, (off + w) // 128)
                                        if classes[rb, cb] == MASK]
                            nc.tensor.matmul(
                                sps[:, 0:w],
                                QT[hq:hq + 64, hp, rb * 128:(rb + 1) * 128],
                                KT[hq:hq + 64, hp, off:off + w],
                                start=True, stop=(len(mask_cbs) == 0))
                            for idx, cb in enumerate(mask_cbs):
                                nc.tensor.matmul(
                                    sps[:, cb * 128 - off:(cb + 1) * 128 - off],
                                    negI[:], mask_t[:, rb, cb * 128:(cb + 1) * 128],
                                    start=False, stop=(idx == len(mask_cbs) - 1))
                            zt = p2sm.tile([128, 1], f32, tag="zt")
                            nc.scalar.activation(pn[:, off:off + w], sps[:, 0:w], Exp,
                                                 accum_out=zt[:])
                            zts.append(zt)
                        if len(zts) == 2:
                            ztot = p2sm.tile([128, 1], f32, tag="ztot")
                            nc.vector.tensor_add(out=ztot[:], in0=zts[0][:], in1=zts[1][:])
                        else:
                            ztot = zts[0]
                        lnz = p2sm.tile([128, 1], f32, tag="lnz")
                        nc.scalar.activation(lnz[:], ztot[:], Ln)
                        nc.vector.tensor_scalar_mul(neglnz[:, rb:rb + 1], lnz[:], -1.0)
                        for (sps, off, w) in sps_list:
                            nc.scalar.activation(pn[:, off:off + w], sps[:, 0:w], Exp,
                                                 bias=neglnz[:, rb:rb + 1])
                        nc.sync.dma_start(
                            out=p_d[h, int(offs[rb]):int(offs[rb]) + 128 * cm * 128]
                            .rearrange("(p c) -> p c", c=cm * 128),
                            in_=pn[:, 0:cm * 128])
                    # ---- -lnZ to free layout [1, S] ----
                    zrow = p2sm.tile([1, S], f32r, tag="zrow")
                    for rb in range(NBLK):
                        zr = psT.tile([1, 128], f32r, tag="ST")
                        nc.tensor.transpose(zr[:], neglnz[:, rb:rb + 1], ident[:])
                        nc.vector.tensor_copy(out=zrow[0:1, rb * 128:(rb + 1) * 128],
                                              in_=zr[:])
                    # ---- transposed path: S^T -> PT (pre-normalized), PV ----
                    aps = {}
                    for (off, w) in _chunks(NBLK):
                        aps[off] = psA.tile([64, 512], f32, tag="aps",
                                            name=f"aps_h{h}_{off}")
                    first_cb = {off: min(cb for cb in range(NBLK)
                                         if any(cmaxb[rb] > cb for rb in
                                                range(off // 128, (off + w) // 128)))
                                for (off, w) in _chunks(NBLK)}
                    for cb in range(NBLK):
                        rmin = int(rminb[cb])
                        if rmin >= NBLK:
                            continue
                        PT = p2.tile([128, S], f32r, tag="PT")
                        pv_chunks = _rchunks(rmin)
                        for (off, w) in pv_chunks:
                            stps = psT.tile([128, 512], f32, tag="ST")
                            nc.tensor.matmul(
                                stps[:, 0:w],
                                KT[hq:hq + 64, hp, cb * 128:(cb + 1) * 128],
                                QT[hq:hq + 64, hp, off:off + w],
                                start=True, stop=False)
                            for rj in range(off // 128, (off + w) // 128):
                                need = (rj < rmin) or (rj < NBLK and cb < cmaxb[rj]
                                                       and classes[rj, cb] == MASK)
                                if need:
                                    nc.tensor.matmul(
                                        stps[:, rj * 128 - off:(rj + 1) * 128 - off],
                                        mask_t[:, rj, cb * 128:(cb + 1) * 128], negI[:],
                                        start=False, stop=False)
                            nc.tensor.matmul(stps[:, 0:w], ones_row[0:1, 0:128],
                                             zrow[0:1, off:off + w],
                                             start=False, stop=True)
                            nc.scalar.activation(PT[:, off:off + w], stps[:, 0:w], Exp)
                        for (off, w) in pv_chunks:
                            last_cb = max(c2 for c2 in range(NBLK)
                                          if int(rminb[c2]) * 128 < off + w)
                            nc.tensor.matmul(
                                aps[off][:, 0:w], V[:, cb, h * 64:(h + 1) * 64],
                                PT[:, off:off + w],
                                start=(cb == first_cb[off]), stop=(cb == last_cb))
                    for (off, w) in _chunks(NBLK):
                        nc.vector.tensor_copy(out=attnT[:, h, off:off + w],
                                              in_=aps[off][:, 0:w])

                # ---------------- phase 3: output projection ----------------
                for rb in range(NBLK):
                    ot = p2pn.tile([128, D], f32, tag="ot")
                    for ch in range(2):
                        ops = psS.tile([128, 512], f32, tag="S")
                        for h in range(HPC):
                            nc.tensor.matmul(
                                ops[:], attnT[:, h, rb * 128:(rb + 1) * 128],
                                wo_p[:, h, ch * 512:(ch + 1) * 512],
                                start=(h == 0), stop=(h == HPC - 1))
                        nc.scalar.activation(ot[:, ch * 512:(ch + 1) * 512], ops[:], Copy)
                    nc.sync.dma_start(out=o_d[rb * 128:(rb + 1) * 128, :], in_=ot[:])

    nc.finalize()
    return nc, cmaxb, offs, packed


def kernel(q, k, v, mask, Wq, bq, Wk, bk, Wv, bv, Wo, bo):
    q = np.asarray(q, np.float32)
    k = np.asarray(k, np.float32)
    v = np.asarray(v, np.float32)
    mask = np.asarray(mask, np.float32)
    Wq = np.asarray(Wq, np.float32)
    Wk = np.asarray(Wk, np.float32)
    Wv = np.asarray(Wv, np.float32)
    Wo = np.asarray(Wo, np.float32)
    bq = np.asarray(bq, np.float32)
    bk = np.asarray(bk, np.float32)
    bv = np.asarray(bv, np.float32)
    bo = np.asarray(bo, np.float32)

    mask2d = mask.reshape(S, S)
    classes, cmaxb = _classify_mask(mask2d)
    qb, kb, vb = bool(bq.any()), bool(bk.any()), bool(bv.any())
    nc, cmaxb, offs, packed = _build_program(classes.tobytes(), qb, kb, vb)

    inv8 = 1.0 / np.sqrt(HD)
    in_maps = []
    for c in range(NCORES):
        b, g = c // 2, c % 2
        sl = slice(g * DG, (g + 1) * DG)
        in_maps.append({
            "xq": q[b], "xk": k[b], "xv": v[b], "mask": mask2d,
            "wq": np.ascontiguousarray(Wq[:, sl]) * inv8,
            "wk": np.ascontiguousarray(Wk[:, sl]),
            "wv": np.ascontiguousarray(Wv[:, sl]),
            "wo": np.ascontiguousarray(Wo[sl, :]),
            "bq": (bq[sl] * inv8).reshape(1, DG),
            "bk": bk[sl].reshape(1, DG),
            "bv": bv[sl].reshape(1, DG),
        })

    trace = bool(os.environ.get("BASS_PROFILE"))
    res = run_bass_kernel_spmd(nc, in_maps, list(range(NCORES)), trace=trace)
    STATS["exec_time_ns"] = res.exec_time_ns
    STATS["profile_json"] = getattr(res, "profile_json", None)

    weights = np.zeros((B, H, S, S), np.float32)
    out = np.zeros((B, S, D), np.float32)
    for c in range(NCORES):
        b, g = c // 2, c % 2
        r = res.results[c]
        out[b] += r["o"]
        p = r["p"]
        for hl in range(HPC):
            hrow = p[hl]
            for rb in range(NBLK):
                cm = int(cmaxb[rb])
                blk = hrow[int(offs[rb]):int(offs[rb]) + 128 * cm * 128]
                weights[b, g * HPC + hl, rb * 128:(rb + 1) * 128, 0:cm * 128] = \
                    blk.reshape(128, cm * 128)
    out += bo.reshape(1, 1, D)
    return out, weights


# revision 6
# speedup vs baseline: 1.2466x; 1.2466x over previous
"""Multi-head attention TRN2 kernel, 8-core SPMD.

Sharding: core c -> (batch b = c//2, head-group g = c%2 covering heads g*8..g*8+8).
Per core: QKV projections for its 8 heads, attention, partial output projection.
Host: sums the 2 out-proj partials per batch, scatters packed attention weights.

All value matmuls run in float32r (1 cyc/row at N>=256, ~1.5e-4 rel err).
Inputs are declared float32r in DRAM so plain HWDGE DMAs feed them (no SWDGE casts).
Mask applied via bf16 (-1e12 * I) matmuls into PSUM. Softmax without max-subtraction
(scores are O(1); masked entries underflow to exact 0 like the fp32 reference).
P normalized on DVE (x 1/Z per row); transposed path (PT, for P@V) normalized via a
rank-1 ones x (-lnZ_row) matmul added into S^T before its exp, so P@V needs no rescale.

Mask is triaged on host at 128x128 block granularity: FREE (no mask work),
MASK (mask matmul), SKIP (fully masked: not computed, not stored; host writes 0).
"""
import functools
import os

import numpy as np

import concourse.bacc as bacc
import concourse.mybir as mybir
from concourse.tile import TileContext
from concourse.bass_utils import run_bass_kernel_spmd
from concourse.masks import make_identity

B, S, D, H = 4, 1024, 1024, 16
HD = D // H            # 64
NCORES = 8
HPC = H // 2           # 8 heads per core
DG = HPC * HD          # 512 per-core head dims
NBLK = S // 128        # 8 blocks of 128
f32 = mybir.dt.float32
f32r = mybir.dt.float32r
bf16 = mybir.dt.bfloat16
Exp = mybir.ActivationFunctionType.Exp
Ln = mybir.ActivationFunctionType.Ln
Copy = mybir.ActivationFunctionType.Copy

FREE, MASK, SKIP = 0, 1, 2

STATS = {}  # exec_time_ns etc. for test harness


def _classify_mask(mask2d):
    """Per 128x128 block: FREE (all zero), SKIP (fully masking), MASK (mixed).

    Returns (classes [8,8] int, cmaxb [8] int) with cmaxb monotone nondecreasing;
    non-suffix SKIPs are reclassified as MASK so storage is a per-row prefix.
    """
    m = mask2d.reshape(NBLK, 128, NBLK, 128)
    classes = np.empty((NBLK, NBLK), np.int64)
    for rb in range(NBLK):
        for cb in range(NBLK):
            blk = m[rb, :, cb, :]
            if not blk.any():
                classes[rb, cb] = FREE
            elif float(blk.min()) * 1e12 >= 1e10:
                classes[rb, cb] = SKIP
            else:
                classes[rb, cb] = MASK
    cmaxb = np.empty(NBLK, np.int64)
    prev = 0
    for rb in range(NBLK):
        nonskip = [cb for cb in range(NBLK) if classes[rb, cb] != SKIP]
        cm = (max(nonskip) + 1) if nonskip else 0
        cm = max(cm, prev, 1)  # monotone; compute at least one block
        prev = cm
        cmaxb[rb] = cm
        for cb in range(cm):
            if classes[rb, cb] == SKIP:
                classes[rb, cb] = MASK  # inside stored prefix: mask it to exact 0
    return classes, cmaxb


def _chunks_exact(total):
    """[(off, width)] with 512-wide chunks and an exact-width tail."""
    out = []
    off = 0
    while off < total:
        w = min(512, total - off)
        out.append((off, w))
        off += w
    return out


def _st_chunks(rmin_blk):
    """Exact chunks covering rows [rmin_blk*128, S), split at the 512 boundary."""
    r0 = rmin_blk * 128
    if r0 >= S:
        return []
    if r0 < 512:
        return [(r0, 512 - r0), (512, 512)]
    return [(r0, S - r0)]


@functools.lru_cache(maxsize=4)
def _build_program(classes_key, qbias, kbias, vbias):
    classes = np.frombuffer(classes_key, dtype=np.int64).reshape(NBLK, NBLK)
    cmaxb = np.array([max(cb for cb in range(NBLK) if classes[rb, cb] != SKIP) + 1
                      for rb in range(NBLK)], np.int64)
    # rminb[cb]: first rb whose stored prefix includes cb (monotone cmaxb => nondecreasing)
    rminb = np.array([min([rb for rb in range(NBLK) if cmaxb[rb] > cb] or [NBLK])
                      for cb in range(NBLK)], np.int64)
    offs = np.concatenate([[0], np.cumsum(cmaxb * 128 * 128)])
    packed = int(offs[-1])  # floats per head

    nc = bacc.Bacc(None, target_bir_lowering=False)
    xq_d = nc.declare_dram_parameter("xq", [S, D], f32r, isOutput=False)
    xk_d = nc.declare_dram_parameter("xk", [S, D], f32r, isOutput=False)
    xv_d = nc.declare_dram_parameter("xv", [S, D], f32r, isOutput=False)
    m_d = nc.declare_dram_parameter("mask", [S, S], f32, isOutput=False)
    wq_d = nc.declare_dram_parameter("wq", [D, DG], f32r, isOutput=False)
    wk_d = nc.declare_dram_parameter("wk", [D, DG], f32r, isOutput=False)
    wv_d = nc.declare_dram_parameter("wv", [D, DG], f32r, isOutput=False)
    wo_d = nc.declare_dram_parameter("wo", [DG, D], f32r, isOutput=False)
    bq_d = nc.declare_dram_parameter("bq", [1, DG], f32r, isOutput=False)
    bk_d = nc.declare_dram_parameter("bk", [1, DG], f32r, isOutput=False)
    bv_d = nc.declare_dram_parameter("bv", [1, DG], f32r, isOutput=False)
    p_d = nc.declare_dram_parameter("p", [HPC, packed], f32, isOutput=True)
    o_d = nc.declare_dram_parameter("o", [S, D], f32, isOutput=True)

    with TileContext(nc) as tc:
        with tc.tile_pool(name="const", bufs=1) as constp, \
             tc.tile_pool(name="persist", bufs=1) as pers:
            identF = constp.tile([128, 128], f32)
            make_identity(nc, identF[:])
            ident = constp.tile([128, 128], f32r)
            nc.vector.tensor_copy(out=ident[:], in_=identF[:])
            negI = constp.tile([128, 128], bf16)
            nc.gpsimd.memset(negI[:], 0.0)
            nc.gpsimd.affine_select(out=negI[:], in_=negI[:],
                                    compare_op=mybir.AluOpType.not_equal,
                                    fill=-1e12, base=0, pattern=[[-1, 128]],
                                    channel_multiplier=1)
            onesF = constp.tile([1, 512], f32)
            nc.gpsimd.memset(onesF[:], 1.0)
            ones_row = constp.tile([1, 512], f32r)
            nc.vector.tensor_copy(out=ones_row[:], in_=onesF[:])

            # persistent operands
            QT = pers.tile([128, 4, S], f32r)     # [p, pair j, s]; head h at rows (h%2)*64, pair h//2
            KT = pers.tile([128, 4, S], f32r)
            V = pers.tile([128, NBLK, DG], f32r)  # [s_in, s_blk, dcat]
            mask_t = pers.tile([128, NBLK, S], bf16)
            wo_p = pers.tile([64, HPC, D], f32r)  # [d_in_head, head, dout]
            nc.sync.dma_start(out=wo_p[:],
                              in_=wo_d.rearrange("(h p) n -> p h n", p=64))
            if qbias or kbias or vbias:
                b_rows = pers.tile([1, 3, DG], f32r)
                nc.sync.dma_start(out=b_rows[0:1, 0, :], in_=bq_d[:])
                nc.sync.dma_start(out=b_rows[0:1, 1, :], in_=bk_d[:])
                nc.sync.dma_start(out=b_rows[0:1, 2, :], in_=bv_d[:])

            # ---------------- phase 1: load + transpose + projections ----------------
            with tc.tile_pool(name="p1", bufs=3) as p1, \
                 tc.tile_pool(name="p1m", bufs=1) as p1m, \
                 tc.tile_pool(name="p1w", bufs=1) as p1w, \
                 tc.tile_pool(name="p1xT", bufs=1) as p1xT, \
                 tc.tile_pool(name="ps1", bufs=2, space="PSUM") as ps1:
                # mask: HWDGE f32 load + Pool-engine cast to bf16
                mask_f = p1m.tile([128, NBLK, S], f32, tag="maskf")
                nc.sync.dma_start(out=mask_f[:],
                                  in_=m_d.rearrange("(b p) c -> p b c", p=128))
                for rb in range(NBLK):
                    nc.gpsimd.tensor_copy(out=mask_t[:, rb, :], in_=mask_f[:, rb, :])

                for x_d, w_d, kind, has_b, bidx in [
                        (xq_d, wq_d, "q", qbias, 0),
                        (xk_d, wk_d, "k", kbias, 1),
                        (xv_d, wv_d, "v", vbias, 2)]:
                    w_t = p1w.tile([128, 8, DG], f32r, tag="w")
                    nc.sync.dma_start(out=w_t[:],
                                      in_=w_d.rearrange("(i p) o -> p i o", p=128))
                    xT = p1xT.tile([128, 8, S], f32r, tag="xT")
                    for sb in range(NBLK):
                        x_t = p1.tile([128, D], f32r, tag="x")
                        nc.sync.dma_start(out=x_t[:], in_=x_d[sb * 128:(sb + 1) * 128, :])
                        for ib in range(8):
                            tp = ps1.tile([128, 128], f32r, tag="tp")
                            nc.tensor.transpose(tp[:], x_t[:, ib * 128:(ib + 1) * 128], ident[:])
                            nc.vector.tensor_copy(out=xT[:, ib, sb * 128:(sb + 1) * 128],
                                                  in_=tp[:])
                    if kind in ("q", "k"):
                        dst = QT if kind == "q" else KT
                        for j in range(4):
                            for ch in range(2):
                                ps = ps1.tile([128, 512], f32, tag="proj")
                                for ki in range(8):
                                    nc.tensor.matmul(
                                        ps[:], w_t[:, ki, j * 128:(j + 1) * 128],
                                        xT[:, ki, ch * 512:(ch + 1) * 512],
                                        start=(ki == 0), stop=(ki == 7 and not has_b))
                                if has_b:
                                    nc.tensor.matmul(
                                        ps[:], b_rows[0:1, bidx, j * 128:(j + 1) * 128],
                                        ones_row[0:1, 0:512], start=False, stop=True)
                                nc.vector.tensor_copy(
                                    out=dst[:, j, ch * 512:(ch + 1) * 512], in_=ps[:])
                    else:
                        for sb in range(NBLK):
                            ps = ps1.tile([128, 512], f32, tag="proj")
                            for ki in range(8):
                                nc.tensor.matmul(
                                    ps[:], xT[:, ki, sb * 128:(sb + 1) * 128],
                                    w_t[:, ki, :],
                                    start=(ki == 0), stop=(ki == 7 and not has_b))
                            if has_b:
                                nc.tensor.matmul(ps[:], ones_row[0:1, 0:128],
                                                 b_rows[0:1, bidx, :],
                                                 start=False, stop=True)
                            nc.vector.tensor_copy(out=V[:, sb, :], in_=ps[:])

            # ---------------- phase 2: attention per head ----------------
            with tc.tile_pool(name="p2", bufs=3) as p2, \
                 tc.tile_pool(name="p2pn", bufs=2) as p2pn, \
                 tc.tile_pool(name="p2pu", bufs=3) as p2pu, \
                 tc.tile_pool(name="p2sm", bufs=3) as p2sm, \
                 tc.tile_pool(name="psS", bufs=3, space="PSUM") as psS, \
                 tc.tile_pool(name="psT", bufs=3, space="PSUM") as psT, \
                 tc.tile_pool(name="psA", bufs=2, space="PSUM") as psA:
                attnT = p2.tile([64, HPC, S], f32r, tag="attnT", bufs=1)
                for h in range(HPC):
                    hp, hq = h // 2, (h % 2) * 64
                    neglnz = p2sm.tile([128, NBLK], f32r, tag="neglnz")
                    # ---- natural path: S, Z, normalized P out ----
                    for rb in range(NBLK):
                        cm = int(cmaxb[rb])
                        chs = _chunks_exact(cm * 128)
                        pn = p2pn.tile([128, 1024], f32, tag="pn")
                        pus = []
                        zts = []
                        for (off, w) in chs:
                            sps = psS.tile([128, 512], f32, tag="S")
                            mask_cbs = [cb for cb in range(off // 128, (off + w) // 128)
                                        if classes[rb, cb] == MASK]
                            nc.tensor.matmul(
                                sps[:, 0:w],
                                QT[hq:hq + 64, hp, rb * 128:(rb + 1) * 128],
                                KT[hq:hq + 64, hp, off:off + w],
                                start=True, stop=(len(mask_cbs) == 0))
                            for idx, cb in enumerate(mask_cbs):
                                nc.tensor.matmul(
                                    sps[:, cb * 128 - off:(cb + 1) * 128 - off],
                                    negI[:], mask_t[:, rb, cb * 128:(cb + 1) * 128],
                                    start=False, stop=(idx == len(mask_cbs) - 1))
                            pu = p2pu.tile([128, 512], f32, tag="pu")
                            zt = p2sm.tile([128, 1], f32, tag="zt")
                            nc.scalar.activation(pu[:, 0:w], sps[:, 0:w], Exp,
                                                 accum_out=zt[:])
                            pus.append((pu, off, w))
                            zts.append(zt)
                        if len(zts) == 2:
                            ztot = p2sm.tile([128, 1], f32, tag="ztot")
                            nc.vector.tensor_add(out=ztot[:], in0=zts[0][:], in1=zts[1][:])
                        else:
                            ztot = zts[0]
                        lnz = p2sm.tile([128, 1], f32, tag="lnz")
                        nc.scalar.activation(lnz[:], ztot[:], Ln)
                        nc.vector.tensor_scalar_mul(neglnz[:, rb:rb + 1], lnz[:], -1.0)
                        rz = p2sm.tile([128, 1], f32, tag="rz")
                        nc.vector.reciprocal(rz[:], ztot[:])
                        for (pu, off, w) in pus:
                            nc.vector.tensor_scalar_mul(pn[:, off:off + w],
                                                        pu[:, 0:w], rz[:])
                        nc.sync.dma_start(
                            out=p_d[h, int(offs[rb]):int(offs[rb]) + 128 * cm * 128]
                            .rearrange("(p c) -> p c", c=cm * 128),
                            in_=pn[:, 0:cm * 128])
                    # ---- -lnZ to free layout [1, S] via tiny PE transposes ----
                    zrow = p2sm.tile([1, S], f32r, tag="zrow")
                    for rb in range(NBLK):
                        zr = psT.tile([1, 128], f32r, tag="ST")
                        nc.tensor.transpose(zr[:], neglnz[:, rb:rb + 1], ident[:])
                        nc.vector.tensor_copy(out=zrow[0:1, rb * 128:(rb + 1) * 128],
                                              in_=zr[:])
                    # ---- transposed path: S^T -> PT (pre-normalized) -> PV ----
                    aps = {}
                    for off in (0, 512):
                        aps[off] = psA.tile([64, 512], f32, tag="aps",
                                            name=f"aps_h{h}_{off}")
                    # region accumulation bounds: cbs whose st-chunks touch region off
                    region_cbs = {off: [cb for cb in range(NBLK)
                                        if any(o2 // 512 * 512 == off
                                               for (o2, w2) in _st_chunks(int(rminb[cb])))]
                                  for off in (0, 512)}
                    for cb in range(NBLK):
                        rmin = int(rminb[cb])
                        st_chs = _st_chunks(rmin)
                        if not st_chs:
                            continue
                        PT = p2.tile([128, S], f32r, tag="PT")
                        for (off, w) in st_chs:
                            stps = psT.tile([128, 512], f32, tag="ST")
                            mask_rjs = [rj for rj in range(off // 128, (off + w) // 128)
                                        if cb < cmaxb[rj] and classes[rj, cb] == MASK]
                            nc.tensor.matmul(
                                stps[:, 0:w],
                                KT[hq:hq + 64, hp, cb * 128:(cb + 1) * 128],
                                QT[hq:hq + 64, hp, off:off + w],
                                start=True, stop=False)
                            for rj in mask_rjs:
                                nc.tensor.matmul(
                                    stps[:, rj * 128 - off:(rj + 1) * 128 - off],
                                    mask_t[:, rj, cb * 128:(cb + 1) * 128], negI[:],
                                    start=False, stop=False)
                            nc.tensor.matmul(stps[:, 0:w], ones_row[0:1, 0:128],
                                             zrow[0:1, off:off + w],
                                             start=False, stop=True)
                            nc.scalar.activation(PT[:, off:off + w], stps[:, 0:w], Exp)
                        for (off, w) in st_chs:
                            reg = off // 512 * 512
                            cbs = region_cbs[reg]
                            nc.tensor.matmul(
                                aps[reg][:, off - reg:off - reg + w],
                                V[:, cb, h * 64:(h + 1) * 64],
                                PT[:, off:off + w],
                                start=(cb == cbs[0]), stop=(cb == cbs[-1]))
                    for off in (0, 512):
                        nc.vector.tensor_copy(out=attnT[:, h, off:off + 512],
                                              in_=aps[off][:, :])

                # ---------------- phase 3: output projection ----------------
                for rb in range(NBLK):
                    ot = p2pn.tile([128, D], f32, tag="ot")
                    for ch in range(2):
                        ops = psS.tile([128, 512], f32, tag="S")
                        for h in range(HPC):
                            nc.tensor.matmul(
                                ops[:], attnT[:, h, rb * 128:(rb + 1) * 128],
                                wo_p[:, h, ch * 512:(ch + 1) * 512],
                                start=(h == 0), stop=(h == HPC - 1))
                        nc.scalar.activation(ot[:, ch * 512:(ch + 1) * 512], ops[:], Copy)
                    nc.sync.dma_start(out=o_d[rb * 128:(rb + 1) * 128, :], in_=ot[:])

    nc.finalize()
    return nc, cmaxb, offs, packed


def kernel(q, k, v, mask, Wq, bq, Wk, bk, Wv, bv, Wo, bo):
    q = np.asarray(q, np.float32)
    k = np.asarray(k, np.float32)
    v = np.asarray(v, np.float32)
    mask = np.asarray(mask, np.float32)
    Wq = np.asarray(Wq, np.float32)
    Wk = np.asarray(Wk, np.float32)
    Wv = np.asarray(Wv, np.float32)
    Wo = np.asarray(Wo, np.float32)
    bq = np.asarray(bq, np.float32)
    bk = np.asarray(bk, np.float32)
    bv = np.asarray(bv, np.float32)
    bo = np.asarray(bo, np.float32)

    mask2d = mask.reshape(S, S)
    classes, cmaxb = _classify_mask(mask2d)
    qb, kb, vb = bool(bq.any()), bool(bk.any()), bool(bv.any())
    nc, cmaxb, offs, packed = _build_program(classes.tobytes(), qb, kb, vb)

    inv8 = 1.0 / np.sqrt(HD)
    in_maps = []
    for c in range(NCORES):
        b, g = c // 2, c % 2
        sl = slice(g * DG, (g + 1) * DG)
        in_maps.append({
            "xq": q[b], "xk": k[b], "xv": v[b], "mask": mask2d,
            "wq": np.ascontiguousarray(Wq[:, sl]) * inv8,
            "wk": np.ascontiguousarray(Wk[:, sl]),
            "wv": np.ascontiguousarray(Wv[:, sl]),
            "wo": np.ascontiguousarray(Wo[sl, :]),
            "bq": (bq[sl] * inv8).reshape(1, DG),
            "bk": bk[sl].reshape(1, DG),
            "bv": bv[sl].reshape(1, DG),
        })

    trace = bool(os.environ.get("BASS_PROFILE"))
    res = run_bass_kernel_spmd(nc, in_maps, list(range(NCORES)), trace=trace)
    STATS["exec_time_ns"] = res.exec_time_ns
    STATS["profile_json"] = getattr(res, "profile_json", None)

    weights = np.zeros((B, H, S, S), np.float32)
    out = np.zeros((B, S, D), np.float32)
    for c in range(NCORES):
        b, g = c // 2, c % 2
        r = res.results[c]
        out[b] += r["o"]
        p = r["p"]
        for hl in range(HPC):
            hrow = p[hl]
            for rb in range(NBLK):
                cm = int(cmaxb[rb])
                blk = hrow[int(offs[rb]):int(offs[rb]) + 128 * cm * 128]
                weights[b, g * HPC + hl, rb * 128:(rb + 1) * 128, 0:cm * 128] = \
                    blk.reshape(128, cm * 128)
    out += bo.reshape(1, 1, D)
    return out, weights


# revision 13
# speedup vs baseline: 1.4642x; 1.1745x over previous
"""Multi-head attention TRN2 kernel, 8-core SPMD.

Sharding: core c -> (batch b = c//2, head-group g = c%2 covering heads g*8..g*8+8).
Per core: QKV projections for its 8 heads, attention, partial output projection.
Host: sums the 2 out-proj partials per batch, scatters packed attention weights.

All value matmuls run in float32r (1 cyc/row at N>=256, ~1.5e-4 rel err).
Inputs are declared float32r in DRAM so plain HWDGE DMAs feed them (no SWDGE casts).
Mask applied via bf16 (-1e12 * I) matmuls into PSUM. Softmax without max-subtraction
(scores are O(1); masked entries underflow to exact 0 like the fp32 reference).
P normalized on DVE (x 1/Z per row); transposed path (PT, for P@V) normalized via a
rank-1 ones x (-lnZ_row) matmul added into S^T before its exp, so P@V needs no rescale.

Mask is triaged on host at 128x128 block granularity: FREE (no mask work),
MASK (mask matmul), SKIP (fully masked: not computed, not stored; host writes 0).
"""
import functools
import os

import numpy as np

import concourse.bacc as bacc
import concourse.mybir as mybir
from concourse.tile import TileContext
from concourse.bass_utils import run_bass_kernel_spmd
from concourse.masks import make_identity

B, S, D, H = 4, 1024, 1024, 16
HD = D // H            # 64
NCORES = 8
HPC = H // 2           # 8 heads per core
DG = HPC * HD          # 512 per-core head dims
NBLK = S // 128        # 8 blocks of 128
f32 = mybir.dt.float32
f32r = mybir.dt.float32r
bf16 = mybir.dt.bfloat16
Exp = mybir.ActivationFunctionType.Exp
Ln = mybir.ActivationFunctionType.Ln
Copy = mybir.ActivationFunctionType.Copy

FREE, MASK, SKIP = 0, 1, 2

STATS = {}  # exec_time_ns etc. for test harness


class _Bacc(bacc.Bacc):
    """Bacc that prefers the natural_log_exp_and_others ACT table set.

    The default greedy chooser picks exp_and_others for Exp, then every Ln
    interleaved between Exps forces a ~1.3us ACT_TABLE_LOAD (measured 101
    loads = 130us). One set holds exp+ln+copy, so prefer it for everything.
    """

    def insert_act_table_loads(self):
        import bass_rust as _bass_rust
        from concourse.hw_specs import get_activation_tables
        has_activation = any(
            isinstance(i, mybir.InstActivation)
            for b in self.main_func.blocks
            for i in b.instructions
        )
        if not has_activation:
            return
        # Keep list order (set ids are positional); steal exp/ln/copy from every
        # other set so all our activations resolve to the one combined set.
        steal = {Exp, Ln, Copy}
        tables = [
            (name, fns if name == "natural_log_exp_and_others" else fns - steal)
            for name, fns in get_activation_tables(self.m.arch).items()
        ]
        _bass_rust.insert_act_table_loads(self, tables)


def _classify_mask(mask2d):
    """Per 128x128 block: FREE (all zero), SKIP (fully masking), MASK (mixed).

    Returns (classes [8,8] int, cmaxb [8] int) with cmaxb monotone nondecreasing;
    non-suffix SKIPs are reclassified as MASK so storage is a per-row prefix.
    """
    m = mask2d.reshape(NBLK, 128, NBLK, 128)
    classes = np.empty((NBLK, NBLK), np.int64)
    for rb in range(NBLK):
        for cb in range(NBLK):
            blk = m[rb, :, cb, :]
            if not blk.any():
                classes[rb, cb] = FREE
            elif float(blk.min()) * 1e12 >= 1e10:
                classes[rb, cb] = SKIP
            else:
                classes[rb, cb] = MASK
    cmaxb = np.empty(NBLK, np.int64)
    prev = 0
    for rb in range(NBLK):
        nonskip = [cb for cb in range(NBLK) if classes[rb, cb] != SKIP]
        cm = (max(nonskip) + 1) if nonskip else 0
        cm = max(cm, prev, 1)  # monotone; compute at least one block
        prev = cm
        cmaxb[rb] = cm
        for cb in range(cm):
            if classes[rb, cb] == SKIP:
                classes[rb, cb] = MASK  # inside stored prefix: mask it to exact 0
    return classes, cmaxb


def _chunks_exact(total):
    """[(off, width)] with 512-wide chunks and an exact-width tail."""
    out = []
    off = 0
    while off < total:
        w = min(512, total - off)
        out.append((off, w))
        off += w
    return out


def _st_chunks(rmin_blk):
    """Exact chunks covering rows [rmin_blk*128, S), split at the 512 boundary."""
    r0 = rmin_blk * 128
    if r0 >= S:
        return []
    if r0 < 512:
        return [(r0, 512 - r0), (512, 512)]
    return [(r0, S - r0)]


@functools.lru_cache(maxsize=4)
def _build_program(classes_key, qbias, kbias, vbias):
    classes = np.frombuffer(classes_key, dtype=np.int64).reshape(NBLK, NBLK)
    cmaxb = np.array([max(cb for cb in range(NBLK) if classes[rb, cb] != SKIP) + 1
                      for rb in range(NBLK)], np.int64)
    # rminb[cb]: first rb whose stored prefix includes cb (monotone cmaxb => nondecreasing)
    rminb = np.array([min([rb for rb in range(NBLK) if cmaxb[rb] > cb] or [NBLK])
                      for cb in range(NBLK)], np.int64)
    offs = np.concatenate([[0], np.cumsum(cmaxb * 128 * 128)])
    packed = int(offs[-1])  # floats per head

    nc = _Bacc(None, target_bir_lowering=False)
    xq_d = nc.declare_dram_parameter("xq", [S, D], f32r, isOutput=False)
    xk_d = nc.declare_dram_parameter("xk", [S, D], f32r, isOutput=False)
    xv_d = nc.declare_dram_parameter("xv", [S, D], f32r, isOutput=False)
    m_d = nc.declare_dram_parameter("mask", [S, S], f32, isOutput=False)
    wq_d = nc.declare_dram_parameter("wq", [D, DG], f32r, isOutput=False)
    wk_d = nc.declare_dram_parameter("wk", [D, DG], f32r, isOutput=False)
    wv_d = nc.declare_dram_parameter("wv", [D, DG], f32r, isOutput=False)
    wo_d = nc.declare_dram_parameter("wo", [DG, D], f32r, isOutput=False)
    bq_d = nc.declare_dram_parameter("bq", [1, DG], f32r, isOutput=False)
    bk_d = nc.declare_dram_parameter("bk", [1, DG], f32r, isOutput=False)
    bv_d = nc.declare_dram_parameter("bv", [1, DG], f32r, isOutput=False)
    p_d = nc.declare_dram_parameter("p", [HPC, packed], f32, isOutput=True)
    o_d = nc.declare_dram_parameter("o", [S, D], f32, isOutput=True)

    with TileContext(nc) as tc:
        with tc.tile_pool(name="const", bufs=1) as constp, \
             tc.tile_pool(name="persist", bufs=1) as pers:
            identF = constp.tile([128, 128], f32)
            make_identity(nc, identF[:])
            ident = constp.tile([128, 128], f32r)
            nc.vector.tensor_copy(out=ident[:], in_=identF[:])
            negI = constp.tile([128, 128], bf16)
            nc.gpsimd.memset(negI[:], 0.0)
            nc.gpsimd.affine_select(out=negI[:], in_=negI[:],
                                    compare_op=mybir.AluOpType.not_equal,
                                    fill=-1e12, base=0, pattern=[[-1, 128]],
                                    channel_multiplier=1)
            onesF = constp.tile([1, 512], f32)
            nc.gpsimd.memset(onesF[:], 1.0)
            ones_row = constp.tile([1, 512], f32r)
            nc.vector.tensor_copy(out=ones_row[:], in_=onesF[:])

            # persistent operands
            QT = pers.tile([128, 4, S], f32r)     # [p, pair j, s]; head h at rows (h%2)*64, pair h//2
            KT = pers.tile([128, 4, S], f32r)
            V = pers.tile([128, NBLK, DG], f32r)  # [s_in, s_blk, dcat]
            mask_t = pers.tile([128, NBLK, S], bf16)
            wo_p = pers.tile([64, HPC, D], f32r)  # [d_in_head, head, dout]
            if qbias or kbias or vbias:
                b_rows = pers.tile([1, 3, DG], f32r)
                nc.scalar.dma_start(out=b_rows[0:1, 0, :], in_=bq_d[:])
                nc.scalar.dma_start(out=b_rows[0:1, 1, :], in_=bk_d[:])
                nc.scalar.dma_start(out=b_rows[0:1, 2, :], in_=bv_d[:])

            # ---------------- phase 1: load + transpose + projections ----------------
            with tc.tile_pool(name="p1", bufs=3) as p1, \
                 tc.tile_pool(name="p1m", bufs=1) as p1m, \
                 tc.tile_pool(name="p1w", bufs=1) as p1w, \
                 tc.tile_pool(name="p1xT", bufs=1) as p1xT, \
                 tc.tile_pool(name="ps1", bufs=2, space="PSUM") as ps1:
                # mask: HWDGE f32 load + Pool-engine cast to bf16
                mask_f = p1m.tile([128, NBLK, S], f32, tag="maskf")
                nc.scalar.dma_start(out=mask_f[:],
                                    in_=m_d.rearrange("(b p) c -> p b c", p=128))
                for rb in range(NBLK):
                    nc.gpsimd.tensor_copy(out=mask_t[:, rb, :], in_=mask_f[:, rb, :])
                nc.scalar.dma_start(out=wo_p[:],
                                    in_=wo_d.rearrange("(h p) n -> p h n", p=64))

                for x_d, w_d, kind, has_b, bidx in [
                        (xq_d, wq_d, "q", qbias, 0),
                        (xk_d, wk_d, "k", kbias, 1),
                        (xv_d, wv_d, "v", vbias, 2)]:
                    w_t = p1w.tile([128, 8, DG], f32r, tag="w")
                    nc.sync.dma_start(out=w_t[:],
                                      in_=w_d.rearrange("(i p) o -> p i o", p=128))
                    xT = p1xT.tile([128, 8, S], f32r, tag="xT")
                    for sb in range(NBLK):
                        x_t = p1.tile([128, D], f32r, tag="x")
                        nc.sync.dma_start(out=x_t[:], in_=x_d[sb * 128:(sb + 1) * 128, :])
                        for ib in range(8):
                            tp = ps1.tile([128, 128], f32r, tag="tp")
                            nc.tensor.transpose(tp[:], x_t[:, ib * 128:(ib + 1) * 128], ident[:])
                            nc.vector.tensor_copy(out=xT[:, ib, sb * 128:(sb + 1) * 128],
                                                  in_=tp[:])
                    if kind in ("q", "k"):
                        dst = QT if kind == "q" else KT
                        for j in range(4):
                            for ch in range(2):
                                ps = ps1.tile([128, 512], f32, tag="proj")
                                for ki in range(8):
                                    nc.tensor.matmul(
                                        ps[:], w_t[:, ki, j * 128:(j + 1) * 128],
                                        xT[:, ki, ch * 512:(ch + 1) * 512],
                                        start=(ki == 0), stop=(ki == 7 and not has_b))
                                if has_b:
                                    nc.tensor.matmul(
                                        ps[:], b_rows[0:1, bidx, j * 128:(j + 1) * 128],
                                        ones_row[0:1, 0:512], start=False, stop=True)
                                nc.vector.tensor_copy(
                                    out=dst[:, j, ch * 512:(ch + 1) * 512], in_=ps[:])
                    else:
                        for sb in range(NBLK):
                            ps = ps1.tile([128, 512], f32, tag="proj")
                            for ki in range(8):
                                nc.tensor.matmul(
                                    ps[:], xT[:, ki, sb * 128:(sb + 1) * 128],
                                    w_t[:, ki, :],
                                    start=(ki == 0), stop=(ki == 7 and not has_b))
                            if has_b:
                                nc.tensor.matmul(ps[:], ones_row[0:1, 0:128],
                                                 b_rows[0:1, bidx, :],
                                                 start=False, stop=True)
                            nc.vector.tensor_copy(out=V[:, sb, :], in_=ps[:])

            # ---------------- phase 2: attention per head ----------------
            with tc.tile_pool(name="p2", bufs=3) as p2, \
                 tc.tile_pool(name="p2pn", bufs=2) as p2pn, \
                 tc.tile_pool(name="p2pu", bufs=3) as p2pu, \
                 tc.tile_pool(name="p2sm", bufs=3) as p2sm, \
                 tc.tile_pool(name="psS", bufs=3, space="PSUM") as psS, \
                 tc.tile_pool(name="psT", bufs=3, space="PSUM") as psT, \
                 tc.tile_pool(name="psA", bufs=2, space="PSUM") as psA:
                attnT = p2.tile([64, HPC, S], f32r, tag="attnT", bufs=1)

                def s_path(h):
                    """S matmuls, exp+Z, normalize+DMA P, -lnZ row. Returns zrow."""
                    hp, hq = h // 2, (h % 2) * 64
                    neglnz = p2sm.tile([128, NBLK], f32r, tag="neglnz",
                                       name=f"neglnz_{h}")
                    for rb in range(NBLK):
                        cm = int(cmaxb[rb])
                        chs = _chunks_exact(cm * 128)
                        pn = p2pn.tile([128, 1024], f32, tag="pn")
                        pus = []
                        zts = []
                        for (off, w) in chs:
                            sps = psS.tile([128, 512], f32, tag="S")
                            mask_cbs = [cb for cb in range(off // 128, (off + w) // 128)
                                        if classes[rb, cb] == MASK]
                            nc.tensor.matmul(
                                sps[:, 0:w],
                                QT[hq:hq + 64, hp, rb * 128:(rb + 1) * 128],
                                KT[hq:hq + 64, hp, off:off + w],
                                start=True, stop=(len(mask_cbs) == 0))
                            for idx, cb in enumerate(mask_cbs):
                                nc.tensor.matmul(
                                    sps[:, cb * 128 - off:(cb + 1) * 128 - off],
                                    negI[:], mask_t[:, rb, cb * 128:(cb + 1) * 128],
                                    start=False, stop=(idx == len(mask_cbs) - 1))
                            pu = p2pu.tile([128, 512], f32, tag="pu")
                            zt = p2sm.tile([128, 1], f32, tag="zt")
                            nc.scalar.activation(pu[:, 0:w], sps[:, 0:w], Exp,
                                                 accum_out=zt[:])
                            pus.append((pu, off, w))
                            zts.append(zt)
                        if len(zts) == 2:
                            ztot = p2sm.tile([128, 1], f32, tag="ztot")
                            nc.vector.tensor_add(out=ztot[:], in0=zts[0][:], in1=zts[1][:])
                        else:
                            ztot = zts[0]
                        lnz = p2sm.tile([128, 1], f32, tag="lnz")
                        nc.scalar.activation(lnz[:], ztot[:], Ln)
                        nc.vector.tensor_scalar_mul(neglnz[:, rb:rb + 1], lnz[:], -1.0)
                        rz = p2sm.tile([128, 1], f32, tag="rz")
                        nc.vector.reciprocal(rz[:], ztot[:])
                        for (pu, off, w) in pus:
                            nc.vector.tensor_scalar_mul(pn[:, off:off + w],
                                                        pu[:, 0:w], rz[:])
                        nc.sync.dma_start(
                            out=p_d[h, int(offs[rb]):int(offs[rb]) + 128 * cm * 128]
                            .rearrange("(p c) -> p c", c=cm * 128),
                            in_=pn[:, 0:cm * 128])
                    # -lnZ to free layout [1, S] via tiny PE transposes
                    zrow = p2sm.tile([1, S], f32r, tag="zrow", name=f"zrow_{h}")
                    for rb in range(NBLK):
                        zr = psT.tile([1, 128], f32r, tag="ST", name=f"zr_{h}_{rb}")
                        nc.tensor.transpose(zr[:], neglnz[:, rb:rb + 1], ident[:])
                        nc.vector.tensor_copy(out=zrow[0:1, rb * 128:(rb + 1) * 128],
                                              in_=zr[:])
                    return zrow

                def st_path(h, zrow):
                    """S^T -> PT (pre-normalized) -> PV -> attnT."""
                    hp, hq = h // 2, (h % 2) * 64
                    aps = {}
                    for off in (0, 512):
                        aps[off] = psA.tile([64, 512], f32, tag="aps",
                                            name=f"aps_h{h}_{off}")
                    # region accumulation bounds: cbs whose st-chunks touch region off
                    region_cbs = {off: [cb for cb in range(NBLK)
                                        if any(o2 // 512 * 512 == off
                                               for (o2, w2) in _st_chunks(int(rminb[cb])))]
                                  for off in (0, 512)}
                    for cb in range(NBLK):
                        rmin = int(rminb[cb])
                        st_chs = _st_chunks(rmin)
                        if not st_chs:
                            continue
                        PT = p2.tile([128, S], f32r, tag="PT", name=f"PT_{h}_{cb}")
                        for (off, w) in st_chs:
                            stps = psT.tile([128, 512], f32, tag="ST")
                            mask_rjs = [rj for rj in range(off // 128, (off + w) // 128)
                                        if cb < cmaxb[rj] and classes[rj, cb] == MASK]
                            nc.tensor.matmul(
                                stps[:, 0:w],
                                KT[hq:hq + 64, hp, cb * 128:(cb + 1) * 128],
                                QT[hq:hq + 64, hp, off:off + w],
                                start=True, stop=False)
                            for rj in mask_rjs:
                                nc.tensor.matmul(
                                    stps[:, rj * 128 - off:(rj + 1) * 128 - off],
                                    mask_t[:, rj, cb * 128:(cb + 1) * 128], negI[:],
                                    start=False, stop=False)
                            nc.tensor.matmul(stps[:, 0:w], ones_row[0:1, 0:128],
                                             zrow[0:1, off:off + w],
                                             start=False, stop=True)
                            nc.scalar.activation(PT[:, off:off + w], stps[:, 0:w], Exp)
                        for (off, w) in st_chs:
                            reg = off // 512 * 512
                            cbs = region_cbs[reg]
                            nc.tensor.matmul(
                                aps[reg][:, off - reg:off - reg + w],
                                V[:, cb, h * 64:(h + 1) * 64],
                                PT[:, off:off + w],
                                start=(cb == cbs[0]), stop=(cb == cbs[-1]))
                    for off in (0, 512):
                        nc.vector.tensor_copy(out=attnT[:, h, off:off + 512],
                                              in_=aps[off][:, :])

                # software pipeline: S-path of head h+1 overlaps ST-path of head h
                zrows = {}
                for h in range(HPC + 1):
                    if h < HPC:
                        zrows[h] = s_path(h)
                    if h >= 1:
                        st_path(h - 1, zrows.pop(h - 1))

                # ---------------- phase 3: output projection ----------------
                for rb in range(NBLK):
                    ot = p2pn.tile([128, D], f32, tag="ot")
                    for ch in range(2):
                        ops = psS.tile([128, 512], f32, tag="S")
                        for h in range(HPC):
                            nc.tensor.matmul(
                                ops[:], attnT[:, h, rb * 128:(rb + 1) * 128],
                                wo_p[:, h, ch * 512:(ch + 1) * 512],
                                start=(h == 0), stop=(h == HPC - 1))
                        nc.scalar.activation(ot[:, ch * 512:(ch + 1) * 512], ops[:], Copy)
                    nc.sync.dma_start(out=o_d[rb * 128:(rb + 1) * 128, :], in_=ot[:])

    nc.finalize()
    return nc, cmaxb, offs, packed


def kernel(q, k, v, mask, Wq, bq, Wk, bk, Wv, bv, Wo, bo):
    q = np.asarray(q, np.float32)
    k = np.asarray(k, np.float32)
    v = np.asarray(v, np.float32)
    mask = np.asarray(mask, np.float32)
    Wq = np.asarray(Wq, np.float32)
    Wk = np.asarray(Wk, np.float32)
    Wv = np.asarray(Wv, np.float32)
    Wo = np.asarray(Wo, np.float32)
    bq = np.asarray(bq, np.float32)
    bk = np.asarray(bk, np.float32)
    bv = np.asarray(bv, np.float32)
    bo = np.asarray(bo, np.float32)

    mask2d = mask.reshape(S, S)
    classes, cmaxb = _classify_mask(mask2d)
    qb, kb, vb = bool(bq.any()), bool(bk.any()), bool(bv.any())
    nc, cmaxb, offs, packed = _build_program(classes.tobytes(), qb, kb, vb)

    inv8 = 1.0 / np.sqrt(HD)
    in_maps = []
    for c in range(NCORES):
        b, g = c // 2, c % 2
        sl = slice(g * DG, (g + 1) * DG)
        in_maps.append({
            "xq": q[b], "xk": k[b], "xv": v[b], "mask": mask2d,
            "wq": np.ascontiguousarray(Wq[:, sl]) * inv8,
            "wk": np.ascontiguousarray(Wk[:, sl]),
            "wv": np.ascontiguousarray(Wv[:, sl]),
            "wo": np.ascontiguousarray(Wo[sl, :]),
            "bq": (bq[sl] * inv8).reshape(1, DG),
            "bk": bk[sl].reshape(1, DG),
            "bv": bv[sl].reshape(1, DG),
        })

    trace = bool(os.environ.get("BASS_PROFILE"))
    res = run_bass_kernel_spmd(nc, in_maps, list(range(NCORES)), trace=trace)
    STATS["exec_time_ns"] = res.exec_time_ns
    STATS["profile_json"] = getattr(res, "profile_json", None)

    weights = np.zeros((B, H, S, S), np.float32)
    out = np.zeros((B, S, D), np.float32)
    for c in range(NCORES):
        b, g = c // 2, c % 2
        r = res.results[c]
        out[b] += r["o"]
        p = r["p"]
        for hl in range(HPC):
            hrow = p[hl]
            for rb in range(NBLK):
                cm = int(cmaxb[rb])
                blk = hrow[int(offs[rb]):int(offs[rb]) + 128 * cm * 128]
                weights[b, g * HPC + hl, rb * 128:(rb + 1) * 128, 0:cm * 128] = \
                    blk.reshape(128, cm * 128)
    out += bo.reshape(1, 1, D)
    return out, weights
